# revision 61
# baseline (speedup 1.0000x reference)
"""WaveNet-style gated residual conv layer on 8 Trainium2 NeuronCores.

Sharding: data-parallel over batch (B=8 -> 1 batch element per core).
Within a core the T=32768 sequence splits into two halves ("groups")
g0/g1. All streamed data is fp16 (fp32 psum accumulate).

Dense-pass structure, 3 Y-passes + 0.5 out-passes per sample (vs 5.5
for the block-diagonal pairing). The xx tile stacks x twice: rows 0:64
= x from col c0-16 (one DMA), rows 64:128 = the same rows shifted +8
(one cheap 16-bit engine copy per cell). Slicing that ONE tile at two
offsets yields all three conv taps:

  pass A1 [128 rows] slice j   : [tap0 x(c-16) ; tap1 x(c-8)] (start)
  pass A2 [128 rows] slice j+8 : [  zeros      ; tap2 x(c)  ]
  pass CN [ 80 rows]           : cond ch0:80 @ W_CN          (stop)

Y psum [128, cols] = [tanh-pre 64 | sig-pre 64] for ONE group. Gate via
one 128-wide Tanh activation (per-partition scale/bias):
  t[tanh rows] = tanh(y_t + b_t);  t[sig rows] = w = tanh((y_s+b_s)/2)
  sigma(y_s+b_s) = (w+1)/2  =>  2z = t * (w+1)
One cross-partition tensor_scalar_add (+1) aligns w with t; one aligned
tensor_tensor mult writes 2z into zt. g1 uses swapped weight columns so
its tanh rows land at partitions 64:128 -> zt = [2z_g0 ; 2z_g1] with no
extra moves. skip2 stores 2z (host halves it); W_O is pre-halved so
out = Wout @ z + b_out exactly.

Queue plan per cell: PE runs 12 Y-matmuls then the out-transform of
cell k-2 (its gate retired ~2 cells ago -> no in-order stall); DVE runs
one shift-copy (prefetched 2 cells ahead) + the two gate ops per group;
Pool runs the other group's shift-copy + half-supercell stores; ACT
runs the two gate activations, then out-bias moves (split with DVE).
"""

import numpy as np
from collections import deque
from contextlib import ExitStack

import concourse.bass as bass
import concourse.tile as tile
from concourse import bacc, mybir
from concourse.bass_utils import run_bass_kernel_spmd

B, C_IN, T = 8, 64, 32768
R, KS, DIL, C_COND = 64, 3, 8, 80
H = T // 2              # 16384 columns per group
PAD = (KS - 1) * DIL    # 16 causal left-pad
SC = 4096               # supercell width (DMA granularity, group-local cols)
NSC = H // SC           # 4 supercells
CELL = 1024             # psum-cell width (2 PSUM banks)
CHUNK = 512             # matmul moving free dim (1 PSUM bank, fp32)
NCELL = SC // CELL
F32 = mybir.dt.float32
F16 = mybir.dt.float16
N_CORES = 8
WCOLS = 7 * 128         # A1_g0 A1_g1 A2_g0 A2_g1 CN_g0 CN_g1 O
OUT_DELAY = 2           # cells between gate and out-transform

_cache = {}


def build_module():
    nc = bacc.Bacc(
        "TRN2", target_bir_lowering=False, debug=False, num_devices=N_CORES
    )

    x2 = nc.dram_tensor("x2", [64, T], F16, kind="ExternalInput")
    cnd = nc.dram_tensor("cnd", [80, T], F16, kind="ExternalInput")
    wpack = nc.dram_tensor("wpack", [128, WCOLS], F16, kind="ExternalInput")
    b5 = nc.dram_tensor("b5", [128, 5], F32, kind="ExternalInput")
    out2 = nc.dram_tensor("out2", [128, H], F16, kind="ExternalOutput")
    skip2 = nc.dram_tensor("skip2", [128, H], F16, kind="ExternalOutput")

    AFT = mybir.ActivationFunctionType
    ALU = mybir.AluOpType

    with tile.TileContext(nc) as tc, ExitStack() as ctx:
        const = ctx.enter_context(tc.tile_pool(name="const", bufs=1))
        xxpool = ctx.enter_context(tc.tile_pool(name="xxp", bufs=3))
        cnpool = ctx.enter_context(tc.tile_pool(name="cnp", bufs=3))
        ztpool = ctx.enter_context(tc.tile_pool(name="ztp", bufs=3))
        ospool = ctx.enter_context(tc.tile_pool(name="osp", bufs=3))
        tpool = ctx.enter_context(tc.tile_pool(name="tp", bufs=4))
        mpool = ctx.enter_context(tc.tile_pool(name="mp", bufs=4))
        ypool = ctx.enter_context(
            tc.tile_pool(name="yp", bufs=3, space=bass.MemorySpace.PSUM)
        )
        oppool = ctx.enter_context(
            tc.tile_pool(name="opp", bufs=2, space=bass.MemorySpace.PSUM)
        )

        wsb = const.tile([128, WCOLS], F16)
        bsc = const.tile([128, 5], F32)

        W_A1 = [wsb[:, 0:128], wsb[:, 128:256]]
        W_A2 = [wsb[:, 256:384], wsb[:, 384:512]]
        W_CN = [wsb[0:80, 512:640], wsb[0:80, 640:768]]
        W_O = wsb[:, 768:896]

        xx = [[None, None] for _ in range(NSC)]
        cn = [[None, None] for _ in range(NSC)]

        def issue_loads(s):
            # s==0 splits per cell, interleaved g0/g1, so cell 0 is
            # ready ASAP.
            c0 = s * SC
            XW = SC + 16
            xp = xxpool.tile([128, 2 * XW], F16, tag="xx")
            cp = cnpool.tile([80, 2 * SC], F16, tag="cn")
            for g in (0, 1):
                xx[s][g] = xp[:, g * XW : (g + 1) * XW]
                cn[s][g] = cp[:, g * SC : (g + 1) * SC]
            if s == 0:
                # startup order = first-matmul dependency order: the DMA
                # wire is exclusive, so every byte ahead of a needed
                # operand delays the first PE issue directly.
                nc.vector.memset(xp[0:64, 0:16], 0.0)
                HB = SC // 2
                nc.sync.dma_start(xx[0][0][0:64, 16 : 16 + HB], x2[:, 0:HB])
                nc.sync.dma_start(wsb[:, 0:512], wpack[:, 0:512])
                nc.sync.dma_start(
                    xx[0][1][0:64, 16 : 16 + HB], x2[:, H : H + HB]
                )
                nc.sync.dma_start(xx[0][1][0:64, 0:16], x2[:, H - 16 : H])
                nc.sync.dma_start(cn[0][0][:, 0:HB], cnd[:, 0:HB])
                nc.sync.dma_start(bsc[:], b5[:])
                nc.sync.dma_start(cn[0][1][:, 0:HB], cnd[:, H : H + HB])
                for g in (0, 1):
                    h0 = g * H
                    nc.sync.dma_start(
                        xx[0][g][0:64, 16 + HB : 16 + SC],
                        x2[:, h0 + HB : h0 + SC],
                    )
                    nc.sync.dma_start(
                        cn[0][g][:, HB:SC], cnd[:, h0 + HB : h0 + SC]
                    )
            else:
                # one 3-dim-AP DMA loads both groups' windows: in walks
                # [rows, 2 group-windows (stride H), cols]; out walks
                # [partitions, 2 column blocks, cols]
                nc.sync.dma_start(
                    xp[0:64, :].rearrange("p (b w) -> p b w", b=2),
                    x2.rearrange("r (b h) -> r b h", b=2)[
                        :, :, c0 - 16 : c0 - 16 + XW
                    ],
                )
                nc.sync.dma_start(
                    cp[:, :].rearrange("p (b w) -> p b w", b=2),
                    cnd.rearrange("r (b h) -> r b h", b=2)[:, :, c0 : c0 + SC],
                )

        issue_loads(0)
        # remaining weights (CN, O) ride behind the first supercell's data
        nc.sync.dma_start(wsb[:, 512:WCOLS], wpack[:, 512:WCOLS])
        issue_loads(1)

        NGC = NSC * NCELL  # global cell count

        def issue_copies(k):
            """xx rows 64:128 (x shifted +8) for cell k, from rows 0:64.

            g0's copy rides DVE (4x fp16), g1's the idle Pool engine.
            Issued OUT_DELAY cells ahead of use. Pass A2 slices 8 columns
            past the cell, so each copy covers [q0+8, q0+CELL+8) and the
            supercell's first copy also covers its own head -- adjacent
            copies tile the row exactly (no WAR overlap with pass reads).
            """
            s_, cell_ = divmod(k, NCELL)
            qq = cell_ * CELL
            a = qq if cell_ == 0 else qq + 8
            for g in (0, 1):
                xt = xx[s_][g]
                eng = nc.gpsimd if g == 1 else nc.vector
                eng.tensor_copy(
                    xt[64:128, a : qq + CELL + 8],
                    xt[0:64, a + 8 : qq + CELL + 16],
                )

        issue_copies(0)
        issue_copies(1)

        ob_idx = [0]

        def emit_out(zt_p, os_p, q0_p, s_p, last_p):
            c0p = s_p * SC
            if s_p == NSC - 1 and q0_p >= SC // 2:
                nc.sync.dma_start(
                    skip2[:, c0p + q0_p : c0p + q0_p + CELL],
                    zt_p[:, q0_p : q0_p + CELL],
                )
            elif q0_p + CELL == SC // 2 or last_p:
                a0 = 0 if q0_p + CELL == SC // 2 else SC // 2
                nc.sync.dma_start(
                    skip2[:, c0p + a0 : c0p + q0_p + CELL],
                    zt_p[:, a0 : q0_p + CELL],
                )
            for q in (0, CHUNK):
                op = oppool.tile([128, CHUNK], F32)
                nc.tensor.matmul(
                    op[:, :],
                    W_O,
                    zt_p[:, q0_p + q : q0_p + q + CHUNK],
                    start=True,
                    stop=True,
                )
                if ob_idx[0] % 2 == 0:
                    nc.scalar.activation(
                        os_p[:, q0_p + q : q0_p + q + CHUNK], op[:],
                        AFT.Identity, bias=bsc[:, 4:5],
                    )
                else:
                    nc.vector.tensor_scalar_add(
                        os_p[:, q0_p + q : q0_p + q + CHUNK], op[:], bsc[:, 4:5]
                    )
                ob_idx[0] += 1
            # os stores trail their bias moves; the final supercell streams
            # per cell so the drain tail after the last compute stays short.
            if s_p == NSC - 1 and q0_p >= SC // 2:
                # Pool queue: parallel to SP so a waiting os store never
                # delays the next cell's zt store during the drain.
                nc.gpsimd.dma_start(
                    out2[:, c0p + q0_p : c0p + q0_p + CELL],
                    os_p[:, q0_p : q0_p + CELL],
                )
            elif q0_p + CELL == SC // 2 or last_p:
                a = 0 if q0_p + CELL == SC // 2 else SC // 2
                bnd = q0_p + CELL
                nc.sync.dma_start(
                    out2[:, c0p + a : c0p + bnd], os_p[:, a:bnd]
                )

        pending = deque()

        for s in range(NSC):
            if s + 2 < NSC:
                issue_loads(s + 2)
            zt_sc = ztpool.tile([128, SC], F16)
            os_sc = ospool.tile([128, SC], F16)
            for cell in range(NCELL):
                q0 = cell * CELL
                k = s * NCELL + cell
                if k + OUT_DELAY < NGC:
                    issue_copies(k + OUT_DELAY)
                for g in (0, 1):
                    Y = ypool.tile([128, CELL], F32, tag="y")
                    xt, ct = xx[s][g], cn[s][g]
                    for q in (0, CHUNK):
                        nc.tensor.matmul(
                            Y[:, q : q + CHUNK],
                            W_A1[g],
                            xt[:, q0 + q : q0 + q + CHUNK],
                            start=True,
                            stop=False,
                        )
                    for q in (0, CHUNK):
                        nc.tensor.matmul(
                            Y[:, q : q + CHUNK],
                            W_A2[g],
                            xt[:, q0 + q + 8 : q0 + q + 8 + CHUNK],
                            start=False,
                            stop=False,
                        )
                    for q in (0, CHUNK):
                        nc.tensor.matmul(
                            Y[:, q : q + CHUNK],
                            W_CN[g],
                            ct[:, q0 + q : q0 + q + CHUNK],
                            start=False,
                            stop=True,
                        )
                    t = tpool.tile([128, CELL], F16)
                    nc.scalar.activation(
                        t[:], Y[:], AFT.Tanh,
                        bias=bsc[:, 2 * g + 1 : 2 * g + 2],
                        scale=bsc[:, 2 * g : 2 * g + 1],
                    )
                    m = mpool.tile([128, CELL], F16)
                    if g == 0:
                        # w rows live at 64:128; align them with tanh rows
                        nc.vector.tensor_scalar_add(
                            m[0:64, :], t[64:128, :], 1.0
                        )
                        nc.vector.tensor_tensor(
                            zt_sc[0:64, q0 : q0 + CELL], t[0:64, :], m[0:64, :],
                            ALU.mult,
                        )
                    else:
                        # swapped layout: w rows at 0:64, tanh rows at 64:128
                        nc.vector.tensor_scalar_add(
                            m[64:128, :], t[0:64, :], 1.0
                        )
                        nc.vector.tensor_tensor(
                            zt_sc[64:128, q0 : q0 + CELL], t[64:128, :],
                            m[64:128, :], ALU.mult,
                        )
                if len(pending) >= OUT_DELAY:
                    emit_out(*pending.popleft())
                pending.append((zt_sc, os_sc, q0, s, cell == NCELL - 1))
        while pending:
            emit_out(*pending.popleft())

    nc.compile()
    return nc


def pack_weights(weight_conv, bias_conv, weight_out, bias_out, weight_cond):
    """wpack fp16 [128, WCOLS] and b5 fp32 [128, 5]."""
    wc = weight_conv.astype(np.float32)        # [128 out, 64 in, 3]
    wcd = weight_cond[:, :, 0].astype(np.float32)  # [128 out, 80]
    wo = weight_out[:, :, 0].astype(np.float32)    # [64, 64]

    # colmap: g0 identity, g1 swaps tanh/sig halves
    cm = [np.arange(128), np.concatenate([np.arange(64, 128), np.arange(64)])]
    W_A1 = [np.zeros((128, 128), np.float32) for _ in range(2)]
    W_A2 = [np.zeros((128, 128), np.float32) for _ in range(2)]
    W_CN = [np.zeros((128, 128), np.float32) for _ in range(2)]
    for g in range(2):
        och = cm[g]  # out channel feeding column j is och[j]
        W_A1[g][0:64, :] = wc[och, :, 0].T
        W_A1[g][64:128, :] = wc[och, :, 1].T
        W_A2[g][64:128, :] = wc[och, :, 2].T
        W_CN[g][0:80, :] = wcd[och, :].T
    W_O = np.zeros((128, 128), np.float32)
    W_O[0:64, 0:64] = 0.5 * wo.T
    W_O[64:128, 64:128] = 0.5 * wo.T
    wpack = np.concatenate(
        [W_A1[0], W_A1[1], W_A2[0], W_A2[1], W_CN[0], W_CN[1], W_O], axis=1
    ).astype(np.float16)

    bt = bias_conv[0:64].astype(np.float32)
    bs = bias_conv[64:128].astype(np.float32)
    b5 = np.zeros((128, 5), np.float32)
    b5[0:64, 0] = 1.0
    b5[64:128, 0] = 0.5
    b5[0:64, 1] = bt
    b5[64:128, 1] = 0.5 * bs
    b5[0:64, 2] = 0.5
    b5[64:128, 2] = 1.0
    b5[0:64, 3] = 0.5 * bs
    b5[64:128, 3] = bt
    b5[0:64, 4] = bias_out
    b5[64:128, 4] = bias_out
    return wpack, b5


def pack_core(x_b, cond_b):
    # unsplit layouts: the paired 3-dim-AP loads pick both group windows
    return x_b.astype(np.float16), cond_b.astype(np.float16)


def make_in_maps(x, cond, weight_conv, bias_conv, weight_out, bias_out, weight_cond):
    wpack, b5 = pack_weights(
        weight_conv, bias_conv, weight_out, bias_out, weight_cond
    )
    in_maps = []
    for b in range(B):
        x2, cnd = pack_core(x[b], cond[b])
        in_maps.append({"x2": x2, "cnd": cnd, "wpack": wpack, "b5": b5})
    return in_maps


def unpack_outputs(results):
    output = np.empty((B, R, T), np.float32)
    skip = np.empty((B, R, T), np.float32)
    for b in range(B):
        o2 = results[b]["out2"].astype(np.float32)
        s2 = results[b]["skip2"].astype(np.float32) * 0.5
        output[b, :, :H] = o2[0:64]
        output[b, :, H:] = o2[64:128]
        skip[b, :, :H] = s2[0:64]
        skip[b, :, H:] = s2[64:128]
    return output, skip


def kernel(**inputs):
    inputs = {k: np.asarray(v, dtype=np.float32) for k, v in inputs.items()}
    if "nc" not in _cache:
        _cache["nc"] = build_module()
    nc = _cache["nc"]
    in_maps = make_in_maps(**inputs)
    res = run_bass_kernel_spmd(nc, in_maps, list(range(N_CORES)))
    return unpack_outputs(res.results)


# revision 62
# speedup vs baseline: 1.0040x; 1.0040x over previous
"""WaveNet-style gated residual conv layer on 8 Trainium2 NeuronCores.

Sharding: data-parallel over batch (B=8 -> 1 batch element per core).
Within a core the T=32768 sequence splits into two halves ("groups")
g0/g1. All streamed data is fp16 (fp32 psum accumulate).

Dense-pass structure, 3 Y-passes + 0.5 out-passes per sample (vs 5.5
for the block-diagonal pairing). The xx tile stacks x twice: rows 0:64
= x from col c0-16 (one DMA), rows 64:128 = the same rows shifted +8
(one cheap 16-bit engine copy per cell). Slicing that ONE tile at two
offsets yields all three conv taps:

  pass A1 [128 rows] slice j   : [tap0 x(c-16) ; tap1 x(c-8)] (start)
  pass A2 [128 rows] slice j+8 : [  zeros      ; tap2 x(c)  ]
  pass CN [ 80 rows]           : cond ch0:80 @ W_CN          (stop)

Y psum [128, cols] = [tanh-pre 64 | sig-pre 64] for ONE group. Gate via
one 128-wide Tanh activation (per-partition scale/bias):
  t[tanh rows] = tanh(y_t + b_t);  t[sig rows] = w = tanh((y_s+b_s)/2)
  sigma(y_s+b_s) = (w+1)/2  =>  2z = t * (w+1)
One cross-partition tensor_scalar_add (+1) aligns w with t; one aligned
tensor_tensor mult writes 2z into zt. g1 uses swapped weight columns so
its tanh rows land at partitions 64:128 -> zt = [2z_g0 ; 2z_g1] with no
extra moves. skip2 stores 2z (host halves it); W_O is pre-halved so
out = Wout @ z + b_out exactly.

Queue plan per cell: PE runs 12 Y-matmuls then the out-transform of
cell k-2 (its gate retired ~2 cells ago -> no in-order stall); DVE runs
one shift-copy (prefetched 2 cells ahead) + the two gate ops per group;
Pool runs the other group's shift-copy + half-supercell stores; ACT
runs the two gate activations, then out-bias moves (split with DVE).
"""

import numpy as np
from collections import deque
from contextlib import ExitStack

import concourse.bass as bass
import concourse.tile as tile
from concourse import bacc, mybir
from concourse.bass_utils import run_bass_kernel_spmd

B, C_IN, T = 8, 64, 32768
R, KS, DIL, C_COND = 64, 3, 8, 80
H = T // 2              # 16384 columns per group
PAD = (KS - 1) * DIL    # 16 causal left-pad
SC = 4096               # supercell width (DMA granularity, group-local cols)
NSC = H // SC           # 4 supercells
CELL = 1024             # psum-cell width (2 PSUM banks)
CHUNK = 512             # matmul moving free dim (1 PSUM bank, fp32)
NCELL = SC // CELL
F32 = mybir.dt.float32
F16 = mybir.dt.float16
N_CORES = 8
WCOLS = 7 * 128         # A1_g0 A1_g1 A2_g0 A2_g1 CN_g0 CN_g1 O
OUT_DELAY = 2           # cells between gate and out-transform

_cache = {}


def build_module():
    nc = bacc.Bacc(
        "TRN2", target_bir_lowering=False, debug=False, num_devices=N_CORES
    )

    x2 = nc.dram_tensor("x2", [64, T], F16, kind="ExternalInput")
    cnd = nc.dram_tensor("cnd", [80, T], F16, kind="ExternalInput")
    wpack = nc.dram_tensor("wpack", [128, WCOLS], F16, kind="ExternalInput")
    b5 = nc.dram_tensor("b5", [128, 5], F32, kind="ExternalInput")
    out2 = nc.dram_tensor("out2", [128, H], F16, kind="ExternalOutput")
    skip2 = nc.dram_tensor("skip2", [128, H], F16, kind="ExternalOutput")

    AFT = mybir.ActivationFunctionType
    ALU = mybir.AluOpType

    with tile.TileContext(nc) as tc, ExitStack() as ctx:
        const = ctx.enter_context(tc.tile_pool(name="const", bufs=1))
        xxpool = ctx.enter_context(tc.tile_pool(name="xxp", bufs=3))
        cnpool = ctx.enter_context(tc.tile_pool(name="cnp", bufs=3))
        ztpool = ctx.enter_context(tc.tile_pool(name="ztp", bufs=3))
        ospool = ctx.enter_context(tc.tile_pool(name="osp", bufs=3))
        tpool = ctx.enter_context(tc.tile_pool(name="tp", bufs=4))
        mpool = ctx.enter_context(tc.tile_pool(name="mp", bufs=4))
        ypool = ctx.enter_context(
            tc.tile_pool(name="yp", bufs=3, space=bass.MemorySpace.PSUM)
        )
        oppool = ctx.enter_context(
            tc.tile_pool(name="opp", bufs=2, space=bass.MemorySpace.PSUM)
        )

        wsb = const.tile([128, WCOLS], F16)
        bsc = const.tile([128, 5], F32)

        W_A1 = [wsb[:, 0:128], wsb[:, 128:256]]
        W_A2 = [wsb[:, 256:384], wsb[:, 384:512]]
        W_CN = [wsb[0:80, 512:640], wsb[0:80, 640:768]]
        W_O = wsb[:, 768:896]

        xx = [[None, None] for _ in range(NSC)]
        cn = [[None, None] for _ in range(NSC)]

        def issue_loads(s):
            # s==0 splits per cell, interleaved g0/g1, so cell 0 is
            # ready ASAP.
            c0 = s * SC
            XW = SC + 16
            xp = xxpool.tile([128, 2 * XW], F16, tag="xx")
            cp = cnpool.tile([80, 2 * SC], F16, tag="cn")
            for g in (0, 1):
                xx[s][g] = xp[:, g * XW : (g + 1) * XW]
                cn[s][g] = cp[:, g * SC : (g + 1) * SC]
            if s == 0:
                # startup order = first-matmul dependency order: the DMA
                # wire is exclusive, so every byte ahead of a needed
                # operand delays the first PE issue directly.
                nc.vector.memset(xp[0:64, 0:16], 0.0)
                HB = SC // 2
                nc.sync.dma_start(xx[0][0][0:64, 16 : 16 + HB], x2[:, 0:HB])
                nc.sync.dma_start(wsb[:, 0:512], wpack[:, 0:512])
                nc.sync.dma_start(
                    xx[0][1][0:64, 16 : 16 + HB], x2[:, H : H + HB]
                )
                nc.sync.dma_start(xx[0][1][0:64, 0:16], x2[:, H - 16 : H])
                nc.sync.dma_start(cn[0][0][:, 0:HB], cnd[:, 0:HB])
                nc.sync.dma_start(bsc[:], b5[:])
                nc.sync.dma_start(cn[0][1][:, 0:HB], cnd[:, H : H + HB])
                for g in (0, 1):
                    h0 = g * H
                    nc.sync.dma_start(
                        xx[0][g][0:64, 16 + HB : 16 + SC],
                        x2[:, h0 + HB : h0 + SC],
                    )
                    nc.sync.dma_start(
                        cn[0][g][:, HB:SC], cnd[:, h0 + HB : h0 + SC]
                    )
            else:
                # one 3-dim-AP DMA loads both groups' windows: in walks
                # [rows, 2 group-windows (stride H), cols]; out walks
                # [partitions, 2 column blocks, cols]
                nc.sync.dma_start(
                    xp[0:64, :].rearrange("p (b w) -> p b w", b=2),
                    x2.rearrange("r (b h) -> r b h", b=2)[
                        :, :, c0 - 16 : c0 - 16 + XW
                    ],
                )
                nc.sync.dma_start(
                    cp[:, :].rearrange("p (b w) -> p b w", b=2),
                    cnd.rearrange("r (b h) -> r b h", b=2)[:, :, c0 : c0 + SC],
                )

        issue_loads(0)
        # remaining weights (CN, O) ride behind the first supercell's data
        nc.sync.dma_start(wsb[:, 512:WCOLS], wpack[:, 512:WCOLS])
        issue_loads(1)

        NGC = NSC * NCELL  # global cell count

        def issue_copies(k):
            """xx rows 64:128 (x shifted +8) for cell k, from rows 0:64.

            g0's copy rides DVE (4x fp16), g1's the idle Pool engine.
            Issued OUT_DELAY cells ahead of use. Pass A2 slices 8 columns
            past the cell, so each copy covers [q0+8, q0+CELL+8) and the
            supercell's first copy also covers its own head -- adjacent
            copies tile the row exactly (no WAR overlap with pass reads).
            """
            s_, cell_ = divmod(k, NCELL)
            qq = cell_ * CELL
            a = qq if cell_ == 0 else qq + 8
            for g in (0, 1):
                xt = xx[s_][g]
                eng = nc.gpsimd if g == 1 else nc.vector
                eng.tensor_copy(
                    xt[64:128, a : qq + CELL + 8],
                    xt[0:64, a + 8 : qq + CELL + 16],
                )

        issue_copies(0)
        issue_copies(1)

        ob_idx = [0]

        def emit_out(zt_p, os_p, q0_p, s_p, last_p):
            c0p = s_p * SC
            if s_p == NSC - 1 and q0_p >= SC // 2:
                nc.sync.dma_start(
                    skip2[:, c0p + q0_p : c0p + q0_p + CELL],
                    zt_p[:, q0_p : q0_p + CELL],
                )
            elif q0_p + CELL == SC // 2 or last_p:
                a0 = 0 if q0_p + CELL == SC // 2 else SC // 2
                nc.sync.dma_start(
                    skip2[:, c0p + a0 : c0p + q0_p + CELL],
                    zt_p[:, a0 : q0_p + CELL],
                )
            for q in (0, CHUNK):
                op = oppool.tile([128, CHUNK], F32)
                nc.tensor.matmul(
                    op[:, :],
                    W_O,
                    zt_p[:, q0_p + q : q0_p + q + CHUNK],
                    start=True,
                    stop=True,
                )
                if ob_idx[0] % 2 == 0:
                    nc.scalar.activation(
                        os_p[:, q0_p + q : q0_p + q + CHUNK], op[:],
                        AFT.Identity, bias=bsc[:, 4:5],
                    )
                else:
                    nc.vector.tensor_scalar_add(
                        os_p[:, q0_p + q : q0_p + q + CHUNK], op[:], bsc[:, 4:5]
                    )
                ob_idx[0] += 1
            # os stores trail their bias moves; the final supercell streams
            # per cell so the drain tail after the last compute stays short.
            if s_p == NSC - 1 and q0_p >= SC // 2:
                # Pool queue: parallel to SP so a waiting os store never
                # delays the next cell's zt store during the drain.
                nc.gpsimd.dma_start(
                    out2[:, c0p + q0_p : c0p + q0_p + CELL],
                    os_p[:, q0_p : q0_p + CELL],
                )
            elif q0_p + CELL == SC // 2 or last_p:
                a = 0 if q0_p + CELL == SC // 2 else SC // 2
                bnd = q0_p + CELL
                nc.sync.dma_start(
                    out2[:, c0p + a : c0p + bnd], os_p[:, a:bnd]
                )

        pending = deque()

        for s in range(NSC):
            if s + 2 < NSC:
                issue_loads(s + 2)
            zt_sc = ztpool.tile([128, SC], F16)
            os_sc = ospool.tile([128, SC], F16)
            for cell in range(NCELL):
                q0 = cell * CELL
                k = s * NCELL + cell
                if k + OUT_DELAY < NGC:
                    issue_copies(k + OUT_DELAY)
                # the final cell runs its act+gate in 512-wide halves:
                # the terminal out-transform/store chain starts on half 0
                # ~0.9us earlier, and the drain-idle engines absorb the
                # extra instruction overhead
                halves = (
                    [(0, CELL)] if k < NGC - 1
                    else [(0, CHUNK), (CHUNK, CHUNK)]
                )
                for g in (0, 1):
                    Y = ypool.tile([128, CELL], F32, tag="y")
                    xt, ct = xx[s][g], cn[s][g]
                    for q in (0, CHUNK):
                        nc.tensor.matmul(
                            Y[:, q : q + CHUNK],
                            W_A1[g],
                            xt[:, q0 + q : q0 + q + CHUNK],
                            start=True,
                            stop=False,
                        )
                    for q in (0, CHUNK):
                        nc.tensor.matmul(
                            Y[:, q : q + CHUNK],
                            W_A2[g],
                            xt[:, q0 + q + 8 : q0 + q + 8 + CHUNK],
                            start=False,
                            stop=False,
                        )
                    for q in (0, CHUNK):
                        nc.tensor.matmul(
                            Y[:, q : q + CHUNK],
                            W_CN[g],
                            ct[:, q0 + q : q0 + q + CHUNK],
                            start=False,
                            stop=True,
                        )
                    for qh, hw in halves:
                        t = tpool.tile([128, hw], F16)
                        nc.scalar.activation(
                            t[:], Y[:, qh : qh + hw], AFT.Tanh,
                            bias=bsc[:, 2 * g + 1 : 2 * g + 2],
                            scale=bsc[:, 2 * g : 2 * g + 1],
                        )
                        m = mpool.tile([128, hw], F16)
                        zs = q0 + qh
                        if g == 0:
                            # w rows at 64:128; align them with tanh rows
                            nc.vector.tensor_scalar_add(
                                m[0:64, :], t[64:128, :], 1.0
                            )
                            nc.vector.tensor_tensor(
                                zt_sc[0:64, zs : zs + hw], t[0:64, :],
                                m[0:64, :], ALU.mult,
                            )
                        else:
                            # swapped: w rows at 0:64, tanh rows at 64:128
                            nc.vector.tensor_scalar_add(
                                m[64:128, :], t[0:64, :], 1.0
                            )
                            nc.vector.tensor_tensor(
                                zt_sc[64:128, zs : zs + hw], t[64:128, :],
                                m[64:128, :], ALU.mult,
                            )
                if len(pending) >= OUT_DELAY:
                    emit_out(*pending.popleft())
                pending.append((zt_sc, os_sc, q0, s, cell == NCELL - 1))
        while pending:
            emit_out(*pending.popleft())

    nc.compile()
    return nc


def pack_weights(weight_conv, bias_conv, weight_out, bias_out, weight_cond):
    """wpack fp16 [128, WCOLS] and b5 fp32 [128, 5]."""
    wc = weight_conv.astype(np.float32)        # [128 out, 64 in, 3]
    wcd = weight_cond[:, :, 0].astype(np.float32)  # [128 out, 80]
    wo = weight_out[:, :, 0].astype(np.float32)    # [64, 64]

    # colmap: g0 identity, g1 swaps tanh/sig halves
    cm = [np.arange(128), np.concatenate([np.arange(64, 128), np.arange(64)])]
    W_A1 = [np.zeros((128, 128), np.float32) for _ in range(2)]
    W_A2 = [np.zeros((128, 128), np.float32) for _ in range(2)]
    W_CN = [np.zeros((128, 128), np.float32) for _ in range(2)]
    for g in range(2):
        och = cm[g]  # out channel feeding column j is och[j]
        W_A1[g][0:64, :] = wc[och, :, 0].T
        W_A1[g][64:128, :] = wc[och, :, 1].T
        W_A2[g][64:128, :] = wc[och, :, 2].T
        W_CN[g][0:80, :] = wcd[och, :].T
    W_O = np.zeros((128, 128), np.float32)
    W_O[0:64, 0:64] = 0.5 * wo.T
    W_O[64:128, 64:128] = 0.5 * wo.T
    wpack = np.concatenate(
        [W_A1[0], W_A1[1], W_A2[0], W_A2[1], W_CN[0], W_CN[1], W_O], axis=1
    ).astype(np.float16)

    bt = bias_conv[0:64].astype(np.float32)
    bs = bias_conv[64:128].astype(np.float32)
    b5 = np.zeros((128, 5), np.float32)
    b5[0:64, 0] = 1.0
    b5[64:128, 0] = 0.5
    b5[0:64, 1] = bt
    b5[64:128, 1] = 0.5 * bs
    b5[0:64, 2] = 0.5
    b5[64:128, 2] = 1.0
    b5[0:64, 3] = 0.5 * bs
    b5[64:128, 3] = bt
    b5[0:64, 4] = bias_out
    b5[64:128, 4] = bias_out
    return wpack, b5


def pack_core(x_b, cond_b):
    # unsplit layouts: the paired 3-dim-AP loads pick both group windows
    return x_b.astype(np.float16), cond_b.astype(np.float16)


def make_in_maps(x, cond, weight_conv, bias_conv, weight_out, bias_out, weight_cond):
    wpack, b5 = pack_weights(
        weight_conv, bias_conv, weight_out, bias_out, weight_cond
    )
    in_maps = []
    for b in range(B):
        x2, cnd = pack_core(x[b], cond[b])
        in_maps.append({"x2": x2, "cnd": cnd, "wpack": wpack, "b5": b5})
    return in_maps


def unpack_outputs(results):
    output = np.empty((B, R, T), np.float32)
    skip = np.empty((B, R, T), np.float32)
    for b in range(B):
        o2 = results[b]["out2"].astype(np.float32)
        s2 = results[b]["skip2"].astype(np.float32) * 0.5
        output[b, :, :H] = o2[0:64]
        output[b, :, H:] = o2[64:128]
        skip[b, :, :H] = s2[0:64]
        skip[b, :, H:] = s2[64:128]
    return output, skip


def kernel(**inputs):
    inputs = {k: np.asarray(v, dtype=np.float32) for k, v in inputs.items()}
    if "nc" not in _cache:
        _cache["nc"] = build_module()
    nc = _cache["nc"]
    in_maps = make_in_maps(**inputs)
    res = run_bass_kernel_spmd(nc, in_maps, list(range(N_CORES)))
    return unpack_outputs(res.results)
